# revision 12
# baseline (speedup 1.0000x reference)
"""PointNet++ MSG set-abstraction kernel, sharded over 8 NeuronCores.

Strategy (per sharding hint): data-parallel over the 16 pieces, 2 pieces per
core. Everything runs on the neuron cores:
  - FPS: 1023-step sequential chain. XLA `while` doesn't compile on this
    stack and host has 1 CPU core, so it runs as chunked *unrolled* pmap
    steps (CHUNK steps per dispatch, carry stays device-resident).
  - kNN: matmul-form score (y.x - 0.5|x|^2, exact same ranking as -|y-x|^2)
    + single top-64; K=32 branch takes the first 32 columns.
  - MLP/BN/maxpool: BatchNorm training stats are global over (S, K) ->
    psum across the 8 cores.
  - Output: all_gather on device, single 12.6MB fetch from core 0 (the
    axon tunnel is bandwidth-bound; one stream beats 8 shard fetches).
"""
import numpy as np
import jax
import jax.numpy as jnp
from jax import lax
from functools import partial

N = 65536
P = 16
NP = N // P          # 4096 points per piece
SP = NP // 4         # 1024 centroids per piece
S = P * SP           # 16384 centroids
K_LIST = [32, 64]
IN_CH = 9
EPS = 1e-5
NDEV = 8
PPD = P // NDEV      # pieces per device = 2
FPS_CHUNK = 93       # 1023 = 11 * 93 steps per dispatch


def _fps_steps(xp, dist, last, nsteps):
    """Unrolled FPS steps for one piece. Returns chosen indices [nsteps]."""
    iota = jnp.arange(NP, dtype=jnp.int32)
    picks = []
    for _ in range(nsteps):
        sel = jnp.take_along_axis(xp, last[None, None], axis=0)[0]   # [3]
        d = jnp.sum((xp - sel) ** 2, axis=-1)                        # [NP]
        dist = jnp.minimum(dist, d)
        m = jnp.max(dist)
        # first-occurrence argmax without variadic reduce
        last = jnp.min(jnp.where(dist == m, iota, NP)).astype(jnp.int32)
        picks.append(last)
    return dist, last, jnp.stack(picks)


@partial(jax.pmap, axis_name='d')
def _fps_chunk(xp, dist, last):
    # xp: [PPD, NP, 3], dist: [PPD, NP], last: [PPD]
    f = partial(_fps_steps, nsteps=FPS_CHUNK)
    dist, last, picks = jax.vmap(f)(xp, dist, last)
    return dist, last, picks                                # picks: [PPD, CHUNK]


@partial(jax.pmap, axis_name='d')
def _fwd(xp, feat, new_y, params0, params1):
    # xp: [PPD, NP, 3], feat: [PPD, NP, 9], new_y: [PPD, SP, 3]
    # kNN score via matmul: argmin_p |y - x_p|^2 == argmax_p (y.x_p - 0.5|x_p|^2)
    xn = jnp.sum(xp * xp, axis=-1, keepdims=True)                # [PPD, NP, 1]
    xa = jnp.concatenate([xp, -0.5 * xn], axis=-1)               # [PPD, NP, 4]
    ya = jnp.concatenate([new_y, jnp.ones(new_y.shape[:2] + (1,),
                                          new_y.dtype)], axis=-1)
    score = jnp.einsum('psc,pnc->psn', ya, xa)                   # [PPD, SP, NP]
    idx64 = lax.top_k(score, max(K_LIST))[1]                     # [PPD, SP, 64]
    outs = []
    for K, params in zip(K_LIST, (params0, params1)):
        idx = idx64[:, :, :K]
        gxyz = jax.vmap(lambda a, i: a[i])(xp, idx)              # [PPD,SP,K,3]
        gfeat = jax.vmap(lambda a, i: a[i])(feat, idx)           # [PPD,SP,K,9]
        gx = gxyz - new_y[:, :, None, :]
        h = jnp.concatenate([gfeat, gx], axis=-1).reshape(PPD * SP, K, 12)
        cnt = float(S * K)
        for (W, b, g, be) in params:
            h = jnp.einsum('skc,oc->sko', h, W) + b
            sm = lax.psum(jnp.sum(h, axis=(0, 1)), 'd')
            sq = lax.psum(jnp.sum(h * h, axis=(0, 1)), 'd')
            mean = sm / cnt
            var = sq / cnt - mean * mean
            h = (h - mean) * lax.rsqrt(var + EPS) * g + be
            h = jax.nn.relu(h)
        outs.append(jnp.max(h, axis=1))
    feats = jnp.concatenate(outs, axis=-1).T                     # [192, PPD*SP]
    return lax.all_gather(feats, 'd')                            # [NDEV, 192, PPD*SP]


def kernel(xyz, piece_id, points, params0, params1):
    xyz = np.asarray(xyz)
    piece_id_np = np.asarray(piece_id)
    points = np.asarray(points)

    x = np.ascontiguousarray(xyz[0].T).reshape(P, NP, 3)
    f = np.ascontiguousarray(points[0].T).reshape(P, NP, IN_CH)

    devs = jax.local_devices()[:NDEV]
    xp = x.reshape(NDEV, PPD, NP, 3)
    fp = f.reshape(NDEV, PPD, NP, IN_CH)
    xp_d = jax.device_put_sharded(list(xp), devs)
    fp_d = jax.device_put_sharded(list(fp), devs)
    pflat = tuple(tuple(np.asarray(t) for t in tup) for tup in params0) + \
            tuple(tuple(np.asarray(t) for t in tup) for tup in params1)
    prep = jax.device_put_replicated(pflat, devs)
    p0, p1 = prep[:3], prep[3:]

    # --- FPS on device, chunked-unrolled ---
    dist = jax.device_put_replicated(
        np.full((PPD, NP), np.inf, np.float32), devs)
    last = jax.device_put_replicated(np.zeros((PPD,), np.int32), devs)
    chunks = []
    for _ in range((SP - 1) // FPS_CHUNK):
        dist, last, picks = _fps_chunk(xp_d, dist, last)
        chunks.append(picks)
    local = np.concatenate(
        [np.zeros((NDEV, PPD, 1), np.int32)]
        + [np.asarray(c) for c in chunks], axis=-1)              # [NDEV, PPD, SP]

    piece_base = (np.arange(P, dtype=np.int64) * NP).reshape(NDEV, PPD, 1)
    centroids = (local.astype(np.int64) + piece_base).reshape(S)

    x_flat = x.reshape(N, 3)
    new_xyz_rows = x_flat[centroids]                             # [S, 3]
    new_y = new_xyz_rows.reshape(NDEV, PPD, SP, 3)
    ny_d = jax.device_put_sharded(list(new_y), devs)

    feats = _fwd(xp_d, fp_d, ny_d, p0, p1)
    feats0 = np.asarray(feats[0])                                # [NDEV, 192, PPD*SP]

    new_xyz = new_xyz_rows.T[None].astype(np.float32)            # [1, 3, S]
    new_pid = piece_id_np[0, 0][centroids][None, None, :]        # [1, 1, S]
    new_points = np.concatenate(list(feats0), axis=-1)[None]     # [1, 192, S]
    return new_xyz, new_pid, np.ascontiguousarray(new_points)


# revision 20
# speedup vs baseline: 2.2600x; 2.2600x over previous
"""PointNet++ MSG set-abstraction kernel, sharded over 8 NeuronCores.

Strategy (per sharding hint): data-parallel over the 16 pieces, 2 pieces per
core. Everything runs on the neuron cores:
  - FPS: 1023-step sequential chain. XLA `while` doesn't compile on this
    stack and the host has 1 CPU core, so it runs as chunked *unrolled*
    pmap steps; the carry and the picked indices stay device-resident.
  - kNN: matmul-form score (y.x - 0.5|x|^2; same ranking as -|y-x|^2) +
    a single top-64; the K=32 branch takes the first 32 columns.
  - MLP/BN/maxpool: BatchNorm training stats are global over (S, K) ->
    psum across the 8 cores.
  - Output: features + new_xyz + piece ids are packed into ONE [196, S]
    f32 buffer (ids bitcast), all_gathered and fetched once from core 0 —
    the axon tunnel charges ~90ms per fetch, so one stream wins.
"""
import numpy as np
import jax
import jax.numpy as jnp
from jax import lax
from functools import partial

N = 65536
P = 16
NP = N // P          # 4096 points per piece
SP = NP // 4         # 1024 centroids per piece
S = P * SP           # 16384 centroids
K_LIST = [32, 64]
IN_CH = 9
EPS = 1e-5
NDEV = 8
PPD = P // NDEV      # pieces per device = 2
FPS_CHUNK = 93       # 1023 = 11 * 93 steps per dispatch
NCH = 192            # output feature channels (64 + 128)


def _fps_steps(xp, dist, last, nsteps):
    """Unrolled FPS steps for one piece. Returns picked indices [nsteps]."""
    iota = jnp.arange(NP, dtype=jnp.int32)
    picks = []
    for _ in range(nsteps):
        sel = jnp.take_along_axis(xp, last[None, None], axis=0)[0]   # [3]
        d = jnp.sum((xp - sel) ** 2, axis=-1)                        # [NP]
        dist = jnp.minimum(dist, d)
        m = jnp.max(dist)
        # first-occurrence argmax without variadic reduce
        last = jnp.min(jnp.where(dist == m, iota, NP)).astype(jnp.int32)
        picks.append(last)
    return dist, last, jnp.stack(picks)


@partial(jax.pmap, axis_name='d')
def _fps_chunk(xp, dist, last):
    # xp: [PPD, NP, 3], dist: [PPD, NP], last: [PPD]
    f = partial(_fps_steps, nsteps=FPS_CHUNK)
    dist, last, picks = jax.vmap(f)(xp, dist, last)
    return dist, last, picks                                # picks: [PPD, CHUNK]


@partial(jax.pmap, axis_name='d')
def _finish(xp, feat, picks, params0, params1):
    # xp: [PPD, NP, 3], feat: [PPD, NP, 9],
    # picks: tuple of [PPD, FPS_CHUNK] i32 chunks
    local = jnp.concatenate(
        (jnp.zeros((PPD, 1), jnp.int32),) + picks, axis=-1)      # [PPD, SP]
    new_y = jax.vmap(lambda a, i: a[i])(xp, local)               # [PPD, SP, 3]

    # kNN score via matmul: argmin_p |y - x_p|^2 == argmax_p (y.x_p - 0.5|x_p|^2)
    xn = jnp.sum(xp * xp, axis=-1, keepdims=True)
    xa = jnp.concatenate([xp, -0.5 * xn], axis=-1)               # [PPD, NP, 4]
    ya = jnp.concatenate([new_y, jnp.ones(new_y.shape[:2] + (1,),
                                          new_y.dtype)], axis=-1)
    score = jnp.einsum('psc,pnc->psn', ya, xa)                   # [PPD, SP, NP]
    idx64 = lax.top_k(score, max(K_LIST))[1]                     # [PPD, SP, 64]

    gxyz64 = jax.vmap(lambda a, i: a[i])(xp, idx64)              # [PPD,SP,64,3]
    gfeat64 = jax.vmap(lambda a, i: a[i])(feat, idx64)           # [PPD,SP,64,9]
    gx64 = gxyz64 - new_y[:, :, None, :]
    h64 = jnp.concatenate([gfeat64, gx64], axis=-1)              # [PPD,SP,64,12]
    outs = []
    for K, params in zip(K_LIST, (params0, params1)):
        h = h64[:, :, :K, :].reshape(PPD * SP, K, 12)
        cnt = float(S * K)
        for (W, b, g, be) in params:
            h = jnp.einsum('skc,oc->sko', h, W) + b
            sm = lax.psum(jnp.sum(h, axis=(0, 1)), 'd')
            sq = lax.psum(jnp.sum(h * h, axis=(0, 1)), 'd')
            mean = sm / cnt
            var = sq / cnt - mean * mean
            h = (h - mean) * lax.rsqrt(var + EPS) * g + be
            h = jax.nn.relu(h)
        outs.append(jnp.max(h, axis=1))

    feats = jnp.concatenate(outs, axis=-1).T                     # [NCH, PPD*SP]
    ny_t = new_y.reshape(PPD * SP, 3).T                          # [3, PPD*SP]
    loc_f = lax.bitcast_convert_type(
        local.reshape(1, PPD * SP), jnp.float32)                 # [1, PPD*SP]
    packed = jnp.concatenate([feats, ny_t, loc_f], axis=0)       # [NCH+4, PPD*SP]
    allg = lax.all_gather(packed, 'd')                           # [NDEV, NCH+4, PPD*SP]
    return jnp.transpose(allg, (1, 0, 2)).reshape(NCH + 4, S)


def kernel(xyz, piece_id, points, params0, params1):
    xyz = np.asarray(xyz)
    pid_np = np.asarray(piece_id)
    points = np.asarray(points)

    x = np.ascontiguousarray(xyz[0].T).reshape(P, NP, 3)
    f = np.ascontiguousarray(points[0].T).reshape(P, NP, IN_CH)

    devs = jax.local_devices()[:NDEV]
    xp_d = jax.device_put_sharded(list(x.reshape(NDEV, PPD, NP, 3)), devs)
    fp_d = jax.device_put_sharded(list(f.reshape(NDEV, PPD, NP, IN_CH)), devs)
    pflat = tuple(tuple(np.asarray(t) for t in tup) for tup in params0) + \
            tuple(tuple(np.asarray(t) for t in tup) for tup in params1)
    prep = jax.device_put_replicated(pflat, devs)
    p0, p1 = prep[:3], prep[3:]

    # --- FPS on device, chunked-unrolled; picks stay on device ---
    dist = jax.device_put_replicated(
        np.full((PPD, NP), np.inf, np.float32), devs)
    last = jax.device_put_replicated(np.zeros((PPD,), np.int32), devs)
    chunks = []
    for _ in range((SP - 1) // FPS_CHUNK):
        dist, last, picks = _fps_chunk(xp_d, dist, last)
        chunks.append(picks)

    packed = _finish(xp_d, fp_d, tuple(chunks), p0, p1)
    big = np.asarray(packed[0])                                  # [NCH+4, S] — one fetch

    new_points = big[:NCH][None]                                 # [1, NCH, S]
    new_xyz = big[NCH:NCH + 3][None]                             # [1, 3, S]
    local = big[NCH + 3].view(np.int32).astype(np.int64)         # [S]
    centroids = local + np.repeat(np.arange(P, dtype=np.int64) * NP, SP)
    new_pid = pid_np[0, 0][centroids][None, None, :]             # [1, 1, S], dtype kept
    return new_xyz, new_pid, new_points


# revision 21
# speedup vs baseline: 3.0007x; 1.3277x over previous
"""PointNet++ MSG set-abstraction kernel, sharded over 8 NeuronCores.

Strategy (per sharding hint): data-parallel over the 16 pieces, 2 pieces per
core. Everything runs on the neuron cores:
  - FPS: 1023-step sequential chain. XLA `while` doesn't compile on this
    stack and the host has 1 CPU core, so it runs as chunked *unrolled*
    pmap steps; the carry and the picked indices stay device-resident.
  - kNN: matmul-form score (y.x - 0.5|x|^2; same ranking as -|y-x|^2) +
    a single top-64; the K=32 branch takes the first 32 columns.
  - MLP/BN/maxpool: BatchNorm training stats are global over (S, K) ->
    psum across the 8 cores.
  - Output: features + new_xyz + centroid indices are packed into ONE
    [196, S] f32 buffer (indices bitcast), all_gathered and fetched once
    from core 0 — the axon tunnel charges ~90ms per fetch, so one stream
    wins. piece_id is gathered host-side in its original dtype.
"""
import numpy as np
import jax
import jax.numpy as jnp
from jax import lax
from functools import partial

N = 65536
P = 16
NP = N // P          # 4096 points per piece
SP = NP // 4         # 1024 centroids per piece
S = P * SP           # 16384 centroids
K_LIST = [32, 64]
IN_CH = 9
EPS = 1e-5
NDEV = 8
PPD = P // NDEV      # pieces per device = 2
FPS_CHUNK = 93       # 1023 = 11 * 93 steps per dispatch
NCH = 192            # output feature channels (64 + 128)


def _fps_steps(xp, dist, last, nsteps):
    """Unrolled FPS steps for one piece. Returns picked indices [nsteps]."""
    iota = jnp.arange(NP, dtype=jnp.int32)
    picks = []
    for _ in range(nsteps):
        sel = jnp.take_along_axis(xp, last[None, None], axis=0)[0]   # [3]
        d = jnp.sum((xp - sel) ** 2, axis=-1)                        # [NP]
        dist = jnp.minimum(dist, d)
        m = jnp.max(dist)
        # first-occurrence argmax without variadic reduce
        last = jnp.min(jnp.where(dist == m, iota, NP)).astype(jnp.int32)
        picks.append(last)
    return dist, last, jnp.stack(picks)


@partial(jax.pmap, axis_name='d')
def _fps_chunk(xp, dist, last):
    # xp: [PPD, NP, 3], dist: [PPD, NP], last: [PPD]
    f = partial(_fps_steps, nsteps=FPS_CHUNK)
    dist, last, picks = jax.vmap(f)(xp, dist, last)
    return dist, last, picks                                # picks: [PPD, CHUNK]


@partial(jax.pmap, axis_name='d')
def _finish(xp, feat, picks, params0, params1):
    # xp: [PPD, NP, 3], feat: [PPD, NP, 9],
    # picks: tuple of [PPD, FPS_CHUNK] i32 chunks
    local = jnp.concatenate(
        (jnp.zeros((PPD, 1), jnp.int32),) + picks, axis=-1)      # [PPD, SP]
    new_y = jax.vmap(lambda a, i: a[i])(xp, local)               # [PPD, SP, 3]

    # kNN score via matmul: argmin_p |y - x_p|^2 == argmax_p (y.x_p - 0.5|x_p|^2)
    xn = jnp.sum(xp * xp, axis=-1, keepdims=True)
    xa = jnp.concatenate([xp, -0.5 * xn], axis=-1)               # [PPD, NP, 4]
    ya = jnp.concatenate([new_y, jnp.ones(new_y.shape[:2] + (1,),
                                          new_y.dtype)], axis=-1)
    score = jnp.einsum('psc,pnc->psn', ya, xa)                   # [PPD, SP, NP]
    idx64 = lax.top_k(score, max(K_LIST))[1]                     # [PPD, SP, 64]

    gxyz64 = jax.vmap(lambda a, i: a[i])(xp, idx64)              # [PPD,SP,64,3]
    gfeat64 = jax.vmap(lambda a, i: a[i])(feat, idx64)           # [PPD,SP,64,9]
    gx64 = gxyz64 - new_y[:, :, None, :]
    h64 = jnp.concatenate([gfeat64, gx64], axis=-1)              # [PPD,SP,64,12]
    outs = []
    for K, params in zip(K_LIST, (params0, params1)):
        h = h64[:, :, :K, :].reshape(PPD * SP, K, 12)
        cnt = float(S * K)
        for (W, b, g, be) in params:
            h = jnp.einsum('skc,oc->sko', h, W) + b
            sm = lax.psum(jnp.sum(h, axis=(0, 1)), 'd')
            sq = lax.psum(jnp.sum(h * h, axis=(0, 1)), 'd')
            mean = sm / cnt
            var = sq / cnt - mean * mean
            h = (h - mean) * lax.rsqrt(var + EPS) * g + be
            h = jax.nn.relu(h)
        outs.append(jnp.max(h, axis=1))

    feats = jnp.concatenate(outs, axis=-1).T                     # [NCH, PPD*SP]
    ny_t = new_y.reshape(PPD * SP, 3).T                          # [3, PPD*SP]
    loc_f = lax.bitcast_convert_type(
        local.reshape(1, PPD * SP), jnp.float32)                 # [1, PPD*SP]
    packed = jnp.concatenate([feats, ny_t, loc_f], axis=0)       # [NCH+4, PPD*SP]
    allg = lax.all_gather(packed, 'd')                           # [NDEV, NCH+4, PPD*SP]
    return jnp.transpose(allg, (1, 0, 2)).reshape(NCH + 4, S)


def kernel(xyz, piece_id, points, params0, params1):
    xyz = np.asarray(xyz)
    pid_np = np.asarray(piece_id)
    points = np.asarray(points)

    x = np.ascontiguousarray(xyz[0].T).reshape(P, NP, 3)
    f = np.ascontiguousarray(points[0].T).reshape(P, NP, IN_CH)

    devs = jax.local_devices()[:NDEV]
    xp_d = jax.device_put_sharded(list(x.reshape(NDEV, PPD, NP, 3)), devs)
    fp_d = jax.device_put_sharded(list(f.reshape(NDEV, PPD, NP, IN_CH)), devs)
    pflat = tuple(tuple(np.asarray(t) for t in tup) for tup in params0) + \
            tuple(tuple(np.asarray(t) for t in tup) for tup in params1)
    prep = jax.device_put_replicated(pflat, devs)
    p0, p1 = prep[:3], prep[3:]

    # --- FPS on device, chunked-unrolled; picks stay on device ---
    dist = jax.device_put_replicated(
        np.full((PPD, NP), np.inf, np.float32), devs)
    last = jax.device_put_replicated(np.zeros((PPD,), np.int32), devs)
    chunks = []
    for _ in range((SP - 1) // FPS_CHUNK):
        dist, last, picks = _fps_chunk(xp_d, dist, last)
        chunks.append(picks)

    packed = _finish(xp_d, fp_d, tuple(chunks), p0, p1)
    big = np.asarray(packed[0])                                  # [NCH+4, S] — one fetch

    new_points = big[:NCH][None]                                 # [1, NCH, S]
    new_xyz = big[NCH:NCH + 3][None]                             # [1, 3, S]
    local = big[NCH + 3].view(np.int32).astype(np.int64)         # [S]
    centroids = local + np.repeat(np.arange(P, dtype=np.int64) * NP, SP)
    new_pid = pid_np[0, 0][centroids][None, None, :]             # [1, 1, S], dtype kept
    return new_xyz, new_pid, new_points


# revision 22
# speedup vs baseline: 3.0386x; 1.0126x over previous
"""PointNet++ MSG set-abstraction kernel, sharded over 8 NeuronCores.

Strategy (per sharding hint): data-parallel over the 16 pieces, 2 pieces per
core. Everything runs on the neuron cores:
  - FPS: 1023-step sequential chain. XLA `while` doesn't compile on this
    stack and the host has 1 CPU core, so it runs as chunked *unrolled*
    pmap steps; the carry and the picked indices stay device-resident.
  - kNN: matmul-form score (y.x - 0.5|x|^2; same ranking as -|y-x|^2) +
    a single top-64; the K=32 branch takes the first 32 columns.
  - MLP/BN/maxpool: BatchNorm training stats are global over (S, K) ->
    psum across the 8 cores.
  - Output: features + new_xyz + centroid indices are packed into ONE
    [196, S] f32 buffer (indices bitcast), all_gathered and fetched once
    from core 0 — the axon tunnel charges ~90ms per fetch, so one stream
    wins. piece_id is gathered host-side in its original dtype.
"""
import numpy as np
import jax
import jax.numpy as jnp
from jax import lax
from functools import partial

N = 65536
P = 16
NP = N // P          # 4096 points per piece
SP = NP // 4         # 1024 centroids per piece
S = P * SP           # 16384 centroids
K_LIST = [32, 64]
IN_CH = 9
EPS = 1e-5
NDEV = 8
PPD = P // NDEV      # pieces per device = 2
FPS_CHUNK = 93       # 1023 = 11 * 93 steps per dispatch
NCH = 192            # output feature channels (64 + 128)


def _fps_steps(xp, dist, last, nsteps):
    """Unrolled FPS steps for one piece. Returns picked indices [nsteps]."""
    iota = jnp.arange(NP, dtype=jnp.int32)
    picks = []
    for _ in range(nsteps):
        sel = jnp.take_along_axis(xp, last[None, None], axis=0)[0]   # [3]
        d = jnp.sum((xp - sel) ** 2, axis=-1)                        # [NP]
        dist = jnp.minimum(dist, d)
        m = jnp.max(dist)
        # first-occurrence argmax without variadic reduce
        last = jnp.min(jnp.where(dist == m, iota, NP)).astype(jnp.int32)
        picks.append(last)
    return dist, last, jnp.stack(picks)


@partial(jax.pmap, axis_name='d')
def _fps_chunk(xp, dist, last):
    # xp: [PPD, NP, 3], dist: [PPD, NP], last: [PPD]
    f = partial(_fps_steps, nsteps=FPS_CHUNK)
    dist, last, picks = jax.vmap(f)(xp, dist, last)
    return dist, last, picks                                # picks: [PPD, CHUNK]


@partial(jax.pmap, axis_name='d')
def _finish(xp, feat, picks, params0, params1):
    # xp: [PPD, NP, 3], feat: [PPD, NP, 9],
    # picks: tuple of [PPD, FPS_CHUNK] i32 chunks
    local = jnp.concatenate(
        (jnp.zeros((PPD, 1), jnp.int32),) + picks, axis=-1)      # [PPD, SP]
    new_y = jax.vmap(lambda a, i: a[i])(xp, local)               # [PPD, SP, 3]

    # kNN score via matmul: argmin_p |y - x_p|^2 == argmax_p (y.x_p - 0.5|x_p|^2)
    xn = jnp.sum(xp * xp, axis=-1, keepdims=True)
    xa = jnp.concatenate([xp, -0.5 * xn], axis=-1)               # [PPD, NP, 4]
    ya = jnp.concatenate([new_y, jnp.ones(new_y.shape[:2] + (1,),
                                          new_y.dtype)], axis=-1)
    score = jnp.einsum('psc,pnc->psn', ya, xa)                   # [PPD, SP, NP]
    idx64 = lax.top_k(score, max(K_LIST))[1]                     # [PPD, SP, 64]

    gxyz64 = jax.vmap(lambda a, i: a[i])(xp, idx64)              # [PPD,SP,64,3]
    gfeat64 = jax.vmap(lambda a, i: a[i])(feat, idx64)           # [PPD,SP,64,9]
    gx64 = gxyz64 - new_y[:, :, None, :]
    h64 = jnp.concatenate([gfeat64, gx64], axis=-1)              # [PPD,SP,64,12]
    outs = []
    for K, params in zip(K_LIST, (params0, params1)):
        h = h64[:, :, :K, :].reshape(PPD * SP, K, 12)
        cnt = float(S * K)
        for (W, b, g, be) in params:
            h = jnp.einsum('skc,oc->sko', h, W) + b
            # one collective per layer: [sum(h); sum(h^2)] together
            stats = lax.psum(jnp.concatenate(
                [jnp.sum(h, axis=(0, 1)), jnp.sum(h * h, axis=(0, 1))]), 'd')
            C = h.shape[-1]
            mean = stats[:C] / cnt
            var = stats[C:] / cnt - mean * mean
            h = (h - mean) * lax.rsqrt(var + EPS) * g + be
            h = jax.nn.relu(h)
        outs.append(jnp.max(h, axis=1))

    feats = jnp.concatenate(outs, axis=-1).T                     # [NCH, PPD*SP]
    ny_t = new_y.reshape(PPD * SP, 3).T                          # [3, PPD*SP]
    loc_f = lax.bitcast_convert_type(
        local.reshape(1, PPD * SP), jnp.float32)                 # [1, PPD*SP]
    packed = jnp.concatenate([feats, ny_t, loc_f], axis=0)       # [NCH+4, PPD*SP]
    allg = lax.all_gather(packed, 'd')                           # [NDEV, NCH+4, PPD*SP]
    return jnp.transpose(allg, (1, 0, 2)).reshape(NCH + 4, S)


def kernel(xyz, piece_id, points, params0, params1):
    xyz = np.asarray(xyz)
    pid_np = np.asarray(piece_id)
    points = np.asarray(points)

    x = np.ascontiguousarray(xyz[0].T).reshape(P, NP, 3)
    f = np.ascontiguousarray(points[0].T).reshape(P, NP, IN_CH)

    devs = jax.local_devices()[:NDEV]
    xp_d = jax.device_put_sharded(list(x.reshape(NDEV, PPD, NP, 3)), devs)
    fp_d = jax.device_put_sharded(list(f.reshape(NDEV, PPD, NP, IN_CH)), devs)
    pflat = tuple(tuple(np.asarray(t) for t in tup) for tup in params0) + \
            tuple(tuple(np.asarray(t) for t in tup) for tup in params1)
    prep = jax.device_put_replicated(pflat, devs)
    p0, p1 = prep[:3], prep[3:]

    # --- FPS on device, chunked-unrolled; picks stay on device ---
    dist = jax.device_put_replicated(
        np.full((PPD, NP), np.inf, np.float32), devs)
    last = jax.device_put_replicated(np.zeros((PPD,), np.int32), devs)
    chunks = []
    for _ in range((SP - 1) // FPS_CHUNK):
        dist, last, picks = _fps_chunk(xp_d, dist, last)
        chunks.append(picks)

    packed = _finish(xp_d, fp_d, tuple(chunks), p0, p1)
    big = np.asarray(packed[0])                                  # [NCH+4, S] — one fetch

    new_points = big[:NCH][None]                                 # [1, NCH, S]
    new_xyz = big[NCH:NCH + 3][None]                             # [1, 3, S]
    local = big[NCH + 3].view(np.int32).astype(np.int64)         # [S]
    centroids = local + np.repeat(np.arange(P, dtype=np.int64) * NP, SP)
    new_pid = pid_np[0, 0][centroids][None, None, :]             # [1, 1, S], dtype kept
    return new_xyz, new_pid, new_points
